# revision 7
# baseline (speedup 1.0000x reference)
"""Distributed Trainium2 kernel for conv-augmented attention (ACAT).

Shapes (hardcoded): Q/K/V [4, 8, 2048, 64] f32, conv_q/conv_k [512, 512, 4],
w [2]. Returns (context [4,8,2048,64], attn [4,8,2048,2048]).

Sharding: 8 cores; core i handles batch i//2, t-half i%2 of the conv output,
which corresponds exactly to heads (i%2)*4 .. +4 of that batch (the torch-style
raw reshape maps conv-output rows t to head t//256). No collectives needed.

Core pipeline: conv (f32r matmuls) -> per-head transposed layouts
qctx/kct1 [128, h*l] (rows 0-63 = features, qctx row 64 = -max (written during
attention, provably zero when the scores matmul reads it), kct1 row 64 = ones,
rows 65-127 = zero).  scores S[q,k] (f32r, K=128) -> max (DVE) -> exp+Z (ACT)
-> attn = E*(1/Z) (GpSimd).  Transposed path: ST[k,q] = kct1^T . qctx gets the
-max bias via row 64 of the matmul; exp -> ET fp16; PV matmul against V with a
ones column appended so row 64 of the accumulator is Z_T; context normalized by
Z_T (independently consistent softmax - required because ST rounds differently
than S).
"""
import sys
sys.path.insert(0, '/opt/trn_rl_repo')
import numpy as np

import concourse.bass as bass
import concourse.bacc as bacc
import concourse.mybir as mybir
from concourse.bass_utils import run_bass_kernel_spmd
from concourse.tile import TileContext

F32 = mybir.dt.float32
F32R = mybir.dt.float32r
F16 = mybir.dt.float16
AX = mybir.AxisListType
ALU = mybir.AluOpType
ACTF = mybir.ActivationFunctionType

B, H, L, DK = 4, 8, 2048, 64
D = H * DK
HPC = 4
TPC = 1024
SQW = 1028

SCALE_MODE = "gpsimd"
_NC_CACHE = {}


def build_nc(ntaps):
    nc = bacc.Bacc(None, target_bir_lowering=False)

    sq = nc.declare_dram_parameter("sq", [D, SQW], F32, isOutput=False)
    sk = nc.declare_dram_parameter("sk", [D, SQW], F32, isOutput=False)
    wq = nc.declare_dram_parameter("wq", [D, ntaps * D], F32, isOutput=False)
    wk = nc.declare_dram_parameter("wk", [D, ntaps * D], F32, isOutput=False)
    rq = nc.declare_dram_parameter("rq", [TPC, D], F32, isOutput=False)
    rk = nc.declare_dram_parameter("rk", [TPC, D], F32, isOutput=False)
    vv = nc.declare_dram_parameter("v", [128, HPC * 16 * 65], F32, isOutput=False)
    idn = nc.declare_dram_parameter("ident", [128, 128], F32, isOutput=False)
    attn_o = nc.declare_dram_parameter("attn_o", [HPC, L, L], F32, isOutput=True)
    ctx_o = nc.declare_dram_parameter("ctx_o", [HPC, L, DK], F32, isOutput=True)

    with TileContext(nc) as tc:
        with tc.tile_pool(name="perm", bufs=1) as perm:
            ident = perm.tile([128, 128], F32, tag="ident")
            nc.sync.dma_start(ident[:], idn[:])
            ident16 = perm.tile([128, 128], F16, tag="ident16")
            nc.vector.tensor_copy(ident16[:], ident[:])
            qctx = perm.tile([128, HPC * L], F32R, tag="qctx")
            kct1 = perm.tile([128, HPC * L], F32R, tag="kct1")
            vbf = perm.tile([128, HPC * 16 * 65], F16, tag="vbf")
            ones_f = perm.tile([1, HPC * L], F32, tag="ones")
            nc.gpsimd.memset(qctx[:].bitcast(F32), 0.0)
            nc.gpsimd.memset(kct1[:].bitcast(F32), 0.0)
            nc.gpsimd.memset(ones_f[:], 1.0)
            nc.vector.tensor_copy(kct1[64:65, :], ones_f[:])

            # ---------- phase A/B: per-signal load + round + conv ----------
            with tc.tile_pool(name="stage", bufs=2) as stage, \
                 tc.tile_pool(name="cps", bufs=2, space="PSUM") as cps, \
                 tc.tile_pool(name="cpt", bufs=2, space="PSUM") as cpt, \
                 tc.tile_pool(name="work", bufs=3) as wk_pool, \
                 tc.tile_pool(name="resid", bufs=4) as resid:
                for i4 in range(4):
                    sv = stage.tile([128, 1040], F32, tag="stg_s")
                    nc.sync.dma_start(sv[:], vv[:, i4 * 1040:(i4 + 1) * 1040])
                    nc.vector.tensor_copy(vbf[:, i4 * 1040:(i4 + 1) * 1040], sv[:])

                jobs = [("q", sq, wq, rq, qctx), ("k", sk, wk, rk, kct1)]
                for name, src_s, src_w, rsrc, dst in jobs:
                  with tc.tile_pool(name=f"cperm{name}", bufs=1) as cperm:
                    sig_r = {}
                    w_r = {}
                    for ci in range(4):
                        st = stage.tile([128, SQW], F32, tag="stg_s")
                        nc.sync.dma_start(st[:], src_s[ci * 128:(ci + 1) * 128, :])
                        t = cperm.tile([128, SQW], F32R, tag=f"sr{ci}")
                        nc.vector.tensor_copy(t[:], st[:])
                        sig_r[ci] = t
                        sw = stage.tile([128, ntaps * D], F32, tag="stg_w")
                        nc.sync.dma_start(sw[:], src_w[ci * 128:(ci + 1) * 128, :])
                        tw = cperm.tile([128, ntaps * D], F32R, tag=f"wr{ci}")
                        nc.vector.tensor_copy(tw[:], sw[:])
                        w_r[ci] = tw
                    for it in range(8):
                        ps = cps.tile([128, D], F32, tag="convps")
                        nmm = 4 * ntaps
                        i_mm = 0
                        for ci in range(4):
                            for jj in range(ntaps):
                                nc.tensor.matmul(
                                    ps[:],
                                    sig_r[ci][:, it * 128 + jj:it * 128 + jj + 128],
                                    w_r[ci][:, jj * D:(jj + 1) * D],
                                    start=(i_mm == 0), stop=(i_mm == nmm - 1),
                                )
                                i_mm += 1
                        rt = resid.tile([128, D], F32, tag="rt")
                        nc.sync.dma_start(rt[:], rsrc[it * 128:(it + 1) * 128, :])
                        qc = wk_pool.tile([128, D], F32, tag="qc")
                        nc.vector.tensor_tensor(qc[:], ps[:], rt[:], ALU.add)
                        hh = it // 2
                        dview = dst[0:64, :].rearrange("p (x t g) -> p x t g", t=128, g=8)
                        for g2 in range(4):
                            pt = cpt.tile([128, 128], F32, tag="tp")
                            nc.tensor.transpose(pt[:], qc[:, g2 * 128:(g2 + 1) * 128], ident[:])
                            for sub in range(2):
                                g = g2 * 2 + sub
                                nc.vector.tensor_copy(
                                    dview[:, 2 * hh + (it % 2), :, g],
                                    pt[sub * 64:(sub + 1) * 64, :])

            # ---------- phase C: attention ----------
            with tc.tile_pool(name="ps_s", bufs=2, space="PSUM") as ps_s, \
                 tc.tile_pool(name="ps_st", bufs=2, space="PSUM") as ps_st, \
                 tc.tile_pool(name="ps_c", bufs=2, space="PSUM") as ps_c, \
                 tc.tile_pool(name="sbE", bufs=5) as sbE, \
                 tc.tile_pool(name="sbET", bufs=3) as sbET, \
                 tc.tile_pool(name="sbA", bufs=3) as sbA, \
                 tc.tile_pool(name="sbC", bufs=4) as sbC, \
                 tc.tile_pool(name="stats", bufs=10) as stats:
                for hh in range(HPC):
                    hb = hh * L
                    for g4 in range(4):
                        zbA = stats.tile([128, 4], F32, tag="zbA")
                        zbB = stats.tile([128, 4], F32, tag="zbB")
                        E4 = []
                        for i4 in range(4):
                            qt = g4 * 4 + i4
                            q0 = hb + qt * 128
                            sA = ps_s.tile([128, 1024], F32, tag="s")
                            sB = ps_s.tile([128, 1024], F32, tag="s")
                            for kb in range(2):
                                nc.tensor.matmul(
                                    sA[:, kb * 512:(kb + 1) * 512],
                                    qctx[:, q0:q0 + 128],
                                    kct1[:, hb + kb * 512: hb + (kb + 1) * 512],
                                    start=True, stop=True)
                            for kb in range(2):
                                nc.tensor.matmul(
                                    sB[:, kb * 512:(kb + 1) * 512],
                                    qctx[:, q0:q0 + 128],
                                    kct1[:, hb + 1024 + kb * 512: hb + 1024 + (kb + 1) * 512],
                                    start=True, stop=True)
                            m1 = stats.tile([128, 1], F32, tag="m1")
                            m2 = stats.tile([128, 1], F32, tag="m2")
                            nc.vector.tensor_reduce(m1[:], sA[:], AX.X, ALU.max)
                            nc.vector.tensor_reduce(m2[:], sB[:], AX.X, ALU.max)
                            mm = stats.tile([128, 1], F32, tag="mm")
                            nc.vector.tensor_tensor(mm[:], m1[:], m2[:], ALU.max)
                            negm = stats.tile([128, 1], F16, tag="negm")
                            nc.vector.tensor_scalar_mul(negm[:], mm[:], -1.0)
                            negm32 = stats.tile([128, 1], F32, tag="negm32")
                            nc.vector.tensor_copy(negm32[:], negm[:])
                            ptn = ps_c.tile([1, 128], F16, tag="ctx")
                            nc.tensor.transpose(ptn[:], negm[:], ident16[:])
                            nc.vector.tensor_copy(qctx[64:65, q0:q0 + 128], ptn[:])
                            E = sbE.tile([128, L], F16, tag="E")
                            nc.scalar.activation(E[:, :1024], sA[:], ACTF.Exp,
                                                 bias=negm32[:], scale=1.0,
                                                 accum_out=zbA[:, i4:i4 + 1])
                            nc.scalar.activation(E[:, 1024:], sB[:], ACTF.Exp,
                                                 bias=negm32[:], scale=1.0,
                                                 accum_out=zbB[:, i4:i4 + 1])
                            E4.append(E)
                        rzb = stats.tile([128, 4], F32, tag="rzb")
                        nc.vector.tensor_tensor(rzb[:], zbA[:], zbB[:], ALU.add)
                        nc.vector.reciprocal(rzb[:], rzb[:])
                        for i4 in range(4):
                            qt = g4 * 4 + i4
                            at = sbA.tile([128, L], F32, tag="at")
                            if SCALE_MODE == "gpsimd":
                                nc.gpsimd.tensor_scalar(at[:], E4[i4][:],
                                                        rzb[:, i4:i4 + 1], None, ALU.mult)
                            else:
                                nc.vector.tensor_scalar_mul(at[:, :1024], E4[i4][:, :1024],
                                                            rzb[:, i4:i4 + 1])
                                nc.scalar.activation(at[:, 1024:], E4[i4][:, 1024:],
                                                     ACTF.Copy, scale=rzb[:, i4:i4 + 1])
                            nc.sync.dma_start(attn_o[hh, qt * 128:(qt + 1) * 128, :], at[:])
                        # transposed path for this 512-wide q group
                        qb = hb + g4 * 512
                        ctxT = ps_c.tile([65, 512], F32, tag="ctx")
                        for kc in range(16):
                            stp = ps_st.tile([128, 512], F32, tag="st")
                            nc.tensor.matmul(
                                stp[:],
                                kct1[:, hb + kc * 128: hb + (kc + 1) * 128],
                                qctx[:, qb:qb + 512],
                                start=True, stop=True)
                            ET = sbET.tile([128, 512], F16, tag="ET")
                            nc.scalar.activation(ET[:], stp[:], ACTF.Exp)
                            nc.tensor.matmul(
                                ctxT[:],
                                vbf[:, (hh * 16 + kc) * 65:(hh * 16 + kc + 1) * 65],
                                ET[:],
                                start=(kc == 0), stop=(kc == 15))
                        rzT = sbC.tile([1, 512], F32, tag="rzT")
                        nc.vector.reciprocal(rzT[:], ctxT[64:65, :])
                        rzB = sbC.tile([64, 512], F32, tag="rzB")
                        nc.gpsimd.partition_broadcast(rzB[:], rzT[:])
                        cs = sbC.tile([64, 512], F32, tag="cs")
                        nc.vector.tensor_tensor(cs[:], ctxT[0:64, :], rzB[:], ALU.mult)
                        for i4 in range(4):
                            ct = ps_c.tile([128, 64], F32, tag="ctx")
                            nc.tensor.transpose(ct[:], cs[:, i4 * 128:(i4 + 1) * 128],
                                                ident[:64, :64])
                            cf = sbC.tile([128, 64], F32, tag="cf")
                            nc.scalar.copy(cf[:], ct[:])
                            qt = g4 * 4 + i4
                            nc.sync.dma_start(
                                ctx_o[hh, qt * 128:(qt + 1) * 128, :], cf[:])
    nc.compile()
    return nc


def _get_nc(ntaps):
    if ntaps not in _NC_CACHE:
        _NC_CACHE[ntaps] = build_nc(ntaps)
    return _NC_CACHE[ntaps]


def _prep_inputs(Q, K, V, conv_q, conv_k, w):
    f_s = np.array([2.0, 4.0], dtype=w.dtype)
    ind = int(np.argmax(f_s * w))
    chosen = (2, 4)[ind]
    taps = list(range(chosen))
    ntaps = len(taps)

    ident = np.eye(128, dtype=np.float32)
    in_maps = []
    for core in range(8):
        bi, th = core // 2, core % 2
        t0 = th * TPC
        h0 = HPC * th
        qsig = Q[bi].reshape(D, 2 * TPC)
        ksig = K[bi].reshape(D, 2 * TPC)

        def slc(sig):
            out = np.zeros((D, SQW), dtype=np.float32)
            lo, hi = t0 - 2, t0 + 1026
            s_lo, s_hi = max(lo, 0), min(hi, 2 * TPC)
            out[:, s_lo - lo:s_hi - lo] = sig[:, s_lo:s_hi]
            return out

        wq_h = np.ascontiguousarray(
            conv_q[:, :, taps].transpose(0, 2, 1).reshape(D, ntaps * D)) * np.float32(0.125)
        wk_h = np.ascontiguousarray(
            conv_k[:, :, taps].transpose(0, 2, 1).reshape(D, ntaps * D))
        rq_h = Q[bi].reshape(2 * TPC, D)[t0:t0 + TPC] * np.float32(0.125)
        rk_h = np.ascontiguousarray(K[bi].reshape(2 * TPC, D)[t0:t0 + TPC])
        v4 = V[bi, h0:h0 + HPC].reshape(HPC, 16, 128, DK).transpose(2, 0, 1, 3)
        v_h = np.concatenate(
            [v4, np.ones((128, HPC, 16, 1), dtype=np.float32)], axis=3
        ).reshape(128, HPC * 16 * 65)
        in_maps.append({
            "sq": slc(qsig), "sk": slc(ksig),
            "wq": wq_h.astype(np.float32), "wk": wk_h.astype(np.float32),
            "rq": rq_h.astype(np.float32), "rk": rk_h,
            "v": np.ascontiguousarray(v_h), "ident": ident,
        })
    return in_maps, ntaps


def kernel(Q, K, V, attn_mask, conv_q, conv_k, w, _trace=False):
    Q, K, V = np.asarray(Q), np.asarray(K), np.asarray(V)
    conv_q, conv_k, w = np.asarray(conv_q), np.asarray(conv_k), np.asarray(w)
    in_maps, ntaps = _prep_inputs(Q, K, V, conv_q, conv_k, w)
    nc = _get_nc(ntaps)
    res = run_bass_kernel_spmd(nc, in_maps, core_ids=list(range(8)), trace=_trace)
    context = np.empty((B, H, L, DK), dtype=np.float32)
    attn = np.empty((B, H, L, L), dtype=np.float32)
    for core in range(8):
        bi, th = core // 2, core % 2
        h0 = HPC * th
        attn[bi, h0:h0 + HPC] = res.results[core]["attn_o"]
        context[bi, h0:h0 + HPC] = res.results[core]["ctx_o"]
    kernel._last_exec_time_ns = res.exec_time_ns
    return context, attn


# revision 8
# speedup vs baseline: 3.0867x; 3.0867x over previous
"""Distributed Trainium2 kernel for conv-augmented attention (ACAT).

Shapes (hardcoded): Q/K/V [4, 8, 2048, 64] f32, conv_q/conv_k [512, 512, 4],
w [2]. Returns (context [4,8,2048,64], attn [4,8,2048,2048]).

Sharding: 8 cores; core i handles batch i//2, t-half i%2 of the conv output,
which corresponds exactly to heads (i%2)*4 .. +4 of that batch (the torch-style
raw reshape maps conv-output rows t to head t//256). No collectives needed.

Core pipeline: conv (f32r matmuls) -> per-head transposed layouts
qctx/kct1 [128, h*l] (rows 0-63 = features, qctx row 64 = -max (written during
attention, provably zero when the scores matmul reads it), kct1 row 64 = ones,
rows 65-127 = zero).  scores S[q,k] (f32r, K=128) -> max (DVE) -> exp+Z (ACT)
-> attn = E*(1/Z) (GpSimd).  Transposed path: ST[k,q] = kct1^T . qctx gets the
-max bias via row 64 of the matmul; exp -> ET fp16; PV matmul against V with a
ones column appended so row 64 of the accumulator is Z_T; context normalized by
Z_T (independently consistent softmax - required because ST rounds differently
than S).
"""
import sys
sys.path.insert(0, '/opt/trn_rl_repo')
import numpy as np

import concourse.bass as bass
import concourse.bacc as bacc
import concourse.mybir as mybir
from concourse.bass_utils import run_bass_kernel_spmd
from concourse.tile import TileContext

F32 = mybir.dt.float32
F32R = mybir.dt.float32r
F16 = mybir.dt.float16
AX = mybir.AxisListType
ALU = mybir.AluOpType
ACTF = mybir.ActivationFunctionType

B, H, L, DK = 4, 8, 2048, 64
D = H * DK
HPC = 4
TPC = 1024
SQW = 1028

SCALE_MODE = "split"
_NC_CACHE = {}


def build_nc(ntaps):
    nc = bacc.Bacc(None, target_bir_lowering=False)

    sq = nc.declare_dram_parameter("sq", [D, SQW], F32, isOutput=False)
    sk = nc.declare_dram_parameter("sk", [D, SQW], F32, isOutput=False)
    wq = nc.declare_dram_parameter("wq", [D, ntaps * D], F32, isOutput=False)
    wk = nc.declare_dram_parameter("wk", [D, ntaps * D], F32, isOutput=False)
    rq = nc.declare_dram_parameter("rq", [TPC, D], F32, isOutput=False)
    rk = nc.declare_dram_parameter("rk", [TPC, D], F32, isOutput=False)
    vv = nc.declare_dram_parameter("v", [128, HPC * 16 * 65], F32, isOutput=False)
    idn = nc.declare_dram_parameter("ident", [128, 128], F32, isOutput=False)
    attn_o = nc.declare_dram_parameter("attn_o", [HPC, L, L], F32, isOutput=True)
    ctx_o = nc.declare_dram_parameter("ctx_o", [HPC, L, DK], F32, isOutput=True)

    with TileContext(nc) as tc:
        with tc.tile_pool(name="perm", bufs=1) as perm:
            ident = perm.tile([128, 128], F32, tag="ident")
            nc.sync.dma_start(ident[:], idn[:])
            ident16 = perm.tile([128, 128], F16, tag="ident16")
            nc.vector.tensor_copy(ident16[:], ident[:])
            qctx = perm.tile([128, HPC * L], F32R, tag="qctx")
            kct1 = perm.tile([128, HPC * L], F32R, tag="kct1")
            vbf = perm.tile([128, HPC * 16 * 65], F16, tag="vbf")
            ones_f = perm.tile([1, HPC * L], F32, tag="ones")
            nc.gpsimd.memset(qctx[:].bitcast(F32), 0.0)
            nc.gpsimd.memset(kct1[:].bitcast(F32), 0.0)
            nc.gpsimd.memset(ones_f[:], 1.0)
            nc.vector.tensor_copy(kct1[64:65, :], ones_f[:])

            # ---------- phase A/B: per-signal load + round + conv ----------
            with tc.tile_pool(name="stage", bufs=2) as stage, \
                 tc.tile_pool(name="cps", bufs=2, space="PSUM") as cps, \
                 tc.tile_pool(name="cpt", bufs=2, space="PSUM") as cpt, \
                 tc.tile_pool(name="work", bufs=3) as wk_pool, \
                 tc.tile_pool(name="resid", bufs=4) as resid:
                for i4 in range(4):
                    sv = stage.tile([128, 1040], F32, tag="stg_s")
                    nc.sync.dma_start(sv[:], vv[:, i4 * 1040:(i4 + 1) * 1040])
                    nc.vector.tensor_copy(vbf[:, i4 * 1040:(i4 + 1) * 1040], sv[:])

                jobs = [("q", sq, wq, rq, qctx), ("k", sk, wk, rk, kct1)]
                for name, src_s, src_w, rsrc, dst in jobs:
                  with tc.tile_pool(name=f"cperm{name}", bufs=1) as cperm:
                    sig_r = {}
                    w_r = {}
                    for ci in range(4):
                        st = stage.tile([128, SQW], F32, tag="stg_s")
                        nc.sync.dma_start(st[:], src_s[ci * 128:(ci + 1) * 128, :])
                        t = cperm.tile([128, SQW], F32R, tag=f"sr{ci}")
                        nc.vector.tensor_copy(t[:], st[:])
                        sig_r[ci] = t
                        sw = stage.tile([128, ntaps * D], F32, tag="stg_w")
                        nc.sync.dma_start(sw[:], src_w[ci * 128:(ci + 1) * 128, :])
                        tw = cperm.tile([128, ntaps * D], F32R, tag=f"wr{ci}")
                        nc.vector.tensor_copy(tw[:], sw[:])
                        w_r[ci] = tw
                    for it in range(8):
                        ps = cps.tile([128, D], F32, tag="convps")
                        nmm = 4 * ntaps
                        i_mm = 0
                        for ci in range(4):
                            for jj in range(ntaps):
                                nc.tensor.matmul(
                                    ps[:],
                                    sig_r[ci][:, it * 128 + jj:it * 128 + jj + 128],
                                    w_r[ci][:, jj * D:(jj + 1) * D],
                                    start=(i_mm == 0), stop=(i_mm == nmm - 1),
                                )
                                i_mm += 1
                        rt = resid.tile([128, D], F32, tag="rt")
                        nc.sync.dma_start(rt[:], rsrc[it * 128:(it + 1) * 128, :])
                        qc = wk_pool.tile([128, D], F32, tag="qc")
                        nc.vector.tensor_tensor(qc[:], ps[:], rt[:], ALU.add)
                        hh = it // 2
                        dview = dst[0:64, :].rearrange("p (x t g) -> p x t g", t=128, g=8)
                        for g2 in range(4):
                            pt = cpt.tile([128, 128], F32, tag="tp")
                            nc.tensor.transpose(pt[:], qc[:, g2 * 128:(g2 + 1) * 128], ident[:])
                            for sub in range(2):
                                g = g2 * 2 + sub
                                nc.vector.tensor_copy(
                                    dview[:, 2 * hh + (it % 2), :, g],
                                    pt[sub * 64:(sub + 1) * 64, :])

            # ---------- phase C: attention ----------
            with tc.tile_pool(name="ps_s", bufs=2, space="PSUM") as ps_s, \
                 tc.tile_pool(name="ps_st", bufs=2, space="PSUM") as ps_st, \
                 tc.tile_pool(name="ps_c", bufs=2, space="PSUM") as ps_c, \
                 tc.tile_pool(name="sbE", bufs=5) as sbE, \
                 tc.tile_pool(name="sbET", bufs=3) as sbET, \
                 tc.tile_pool(name="sbA", bufs=3) as sbA, \
                 tc.tile_pool(name="sbC", bufs=4) as sbC, \
                 tc.tile_pool(name="stats", bufs=10) as stats:
                for hh in range(HPC):
                    hb = hh * L
                    for g4 in range(4):
                        zbA = stats.tile([128, 4], F32, tag="zbA")
                        zbB = stats.tile([128, 4], F32, tag="zbB")
                        E4 = []
                        for i4 in range(4):
                            qt = g4 * 4 + i4
                            q0 = hb + qt * 128
                            sA = ps_s.tile([128, 1024], F32, tag="s")
                            sB = ps_s.tile([128, 1024], F32, tag="s")
                            for kb in range(2):
                                nc.tensor.matmul(
                                    sA[:, kb * 512:(kb + 1) * 512],
                                    qctx[:, q0:q0 + 128],
                                    kct1[:, hb + kb * 512: hb + (kb + 1) * 512],
                                    start=True, stop=True)
                            for kb in range(2):
                                nc.tensor.matmul(
                                    sB[:, kb * 512:(kb + 1) * 512],
                                    qctx[:, q0:q0 + 128],
                                    kct1[:, hb + 1024 + kb * 512: hb + 1024 + (kb + 1) * 512],
                                    start=True, stop=True)
                            m1 = stats.tile([128, 1], F32, tag="m1")
                            m2 = stats.tile([128, 1], F32, tag="m2")
                            nc.vector.tensor_reduce(m1[:], sA[:], AX.X, ALU.max)
                            nc.vector.tensor_reduce(m2[:], sB[:], AX.X, ALU.max)
                            mm = stats.tile([128, 1], F32, tag="mm")
                            nc.vector.tensor_tensor(mm[:], m1[:], m2[:], ALU.max)
                            negm = stats.tile([128, 1], F16, tag="negm")
                            nc.vector.tensor_scalar_mul(negm[:], mm[:], -1.0)
                            negm32 = stats.tile([128, 1], F32, tag="negm32")
                            nc.vector.tensor_copy(negm32[:], negm[:])
                            ptn = ps_c.tile([1, 128], F16, tag="ctx")
                            nc.tensor.transpose(ptn[:], negm[:], ident16[:])
                            nc.vector.tensor_copy(qctx[64:65, q0:q0 + 128], ptn[:])
                            E = sbE.tile([128, L], F16, tag="E")
                            nc.scalar.activation(E[:, :1024], sA[:], ACTF.Exp,
                                                 bias=negm32[:], scale=1.0,
                                                 accum_out=zbA[:, i4:i4 + 1])
                            nc.scalar.activation(E[:, 1024:], sB[:], ACTF.Exp,
                                                 bias=negm32[:], scale=1.0,
                                                 accum_out=zbB[:, i4:i4 + 1])
                            E4.append(E)
                        rzb = stats.tile([128, 4], F32, tag="rzb")
                        nc.vector.tensor_tensor(rzb[:], zbA[:], zbB[:], ALU.add)
                        nc.vector.reciprocal(rzb[:], rzb[:])
                        for i4 in range(4):
                            qt = g4 * 4 + i4
                            at = sbA.tile([128, L], F32, tag="at")
                            if SCALE_MODE == "gpsimd":
                                nc.gpsimd.tensor_scalar(at[:], E4[i4][:],
                                                        rzb[:, i4:i4 + 1], None, ALU.mult)
                            else:
                                nc.vector.tensor_scalar_mul(at[:, :1024], E4[i4][:, :1024],
                                                            rzb[:, i4:i4 + 1])
                                nc.scalar.activation(at[:, 1024:], E4[i4][:, 1024:],
                                                     ACTF.Copy, scale=rzb[:, i4:i4 + 1])
                            nc.sync.dma_start(attn_o[hh, qt * 128:(qt + 1) * 128, :], at[:])
                        # transposed path for this 512-wide q group
                        qb = hb + g4 * 512
                        ctxT = ps_c.tile([65, 512], F32, tag="ctx")
                        for kc in range(16):
                            stp = ps_st.tile([128, 512], F32, tag="st")
                            nc.tensor.matmul(
                                stp[:],
                                kct1[:, hb + kc * 128: hb + (kc + 1) * 128],
                                qctx[:, qb:qb + 512],
                                start=True, stop=True)
                            ET = sbET.tile([128, 512], F16, tag="ET")
                            nc.scalar.activation(ET[:], stp[:], ACTF.Exp)
                            nc.tensor.matmul(
                                ctxT[:],
                                vbf[:, (hh * 16 + kc) * 65:(hh * 16 + kc + 1) * 65],
                                ET[:],
                                start=(kc == 0), stop=(kc == 15))
                        rzT = sbC.tile([1, 512], F32, tag="rzT")
                        nc.vector.reciprocal(rzT[:], ctxT[64:65, :])
                        rzB = sbC.tile([64, 512], F32, tag="rzB")
                        nc.gpsimd.partition_broadcast(rzB[:], rzT[:])
                        cs = sbC.tile([64, 512], F32, tag="cs")
                        nc.vector.tensor_tensor(cs[:], ctxT[0:64, :], rzB[:], ALU.mult)
                        for i4 in range(4):
                            ct = ps_c.tile([128, 64], F32, tag="ctx")
                            nc.tensor.transpose(ct[:], cs[:, i4 * 128:(i4 + 1) * 128],
                                                ident[:64, :64])
                            cf = sbC.tile([128, 64], F32, tag="cf")
                            nc.scalar.copy(cf[:], ct[:])
                            qt = g4 * 4 + i4
                            nc.sync.dma_start(
                                ctx_o[hh, qt * 128:(qt + 1) * 128, :], cf[:])
    nc.compile()
    return nc


def _get_nc(ntaps):
    if ntaps not in _NC_CACHE:
        _NC_CACHE[ntaps] = build_nc(ntaps)
    return _NC_CACHE[ntaps]


def _prep_inputs(Q, K, V, conv_q, conv_k, w):
    f_s = np.array([2.0, 4.0], dtype=w.dtype)
    ind = int(np.argmax(f_s * w))
    chosen = (2, 4)[ind]
    taps = list(range(chosen))
    ntaps = len(taps)

    ident = np.eye(128, dtype=np.float32)
    in_maps = []
    for core in range(8):
        bi, th = core // 2, core % 2
        t0 = th * TPC
        h0 = HPC * th
        qsig = Q[bi].reshape(D, 2 * TPC)
        ksig = K[bi].reshape(D, 2 * TPC)

        def slc(sig):
            out = np.zeros((D, SQW), dtype=np.float32)
            lo, hi = t0 - 2, t0 + 1026
            s_lo, s_hi = max(lo, 0), min(hi, 2 * TPC)
            out[:, s_lo - lo:s_hi - lo] = sig[:, s_lo:s_hi]
            return out

        wq_h = np.ascontiguousarray(
            conv_q[:, :, taps].transpose(0, 2, 1).reshape(D, ntaps * D)) * np.float32(0.125)
        wk_h = np.ascontiguousarray(
            conv_k[:, :, taps].transpose(0, 2, 1).reshape(D, ntaps * D))
        rq_h = Q[bi].reshape(2 * TPC, D)[t0:t0 + TPC] * np.float32(0.125)
        rk_h = np.ascontiguousarray(K[bi].reshape(2 * TPC, D)[t0:t0 + TPC])
        v4 = V[bi, h0:h0 + HPC].reshape(HPC, 16, 128, DK).transpose(2, 0, 1, 3)
        v_h = np.concatenate(
            [v4, np.ones((128, HPC, 16, 1), dtype=np.float32)], axis=3
        ).reshape(128, HPC * 16 * 65)
        in_maps.append({
            "sq": slc(qsig), "sk": slc(ksig),
            "wq": wq_h.astype(np.float32), "wk": wk_h.astype(np.float32),
            "rq": rq_h.astype(np.float32), "rk": rk_h,
            "v": np.ascontiguousarray(v_h), "ident": ident,
        })
    return in_maps, ntaps


def kernel(Q, K, V, attn_mask, conv_q, conv_k, w, _trace=False):
    Q, K, V = np.asarray(Q), np.asarray(K), np.asarray(V)
    conv_q, conv_k, w = np.asarray(conv_q), np.asarray(conv_k), np.asarray(w)
    in_maps, ntaps = _prep_inputs(Q, K, V, conv_q, conv_k, w)
    nc = _get_nc(ntaps)
    res = run_bass_kernel_spmd(nc, in_maps, core_ids=list(range(8)), trace=_trace)
    context = np.empty((B, H, L, DK), dtype=np.float32)
    attn = np.empty((B, H, L, L), dtype=np.float32)
    for core in range(8):
        bi, th = core // 2, core % 2
        h0 = HPC * th
        attn[bi, h0:h0 + HPC] = res.results[core]["attn_o"]
        context[bi, h0:h0 + HPC] = res.results[core]["ctx_o"]
    kernel._last_exec_time_ns = res.exec_time_ns
    return context, attn


# revision 10
# speedup vs baseline: 3.1492x; 1.0202x over previous
"""Distributed Trainium2 kernel for conv-augmented attention (ACAT).

Shapes (hardcoded): Q/K/V [4, 8, 2048, 64] f32, conv_q/conv_k [512, 512, 4],
w [2]. Returns (context [4,8,2048,64], attn [4,8,2048,2048]).

Sharding: 8 cores; core i handles batch i//2, t-half i%2 of the conv output,
which corresponds exactly to heads (i%2)*4 .. +4 of that batch (the torch-style
raw reshape maps conv-output rows t to head t//256). No collectives needed.

Core pipeline: conv (f32r matmuls) -> per-head transposed layouts
qctx/kct1 [128, h*l] (rows 0-63 = features, qctx row 64 = -max (written during
attention, provably zero when the scores matmul reads it), kct1 row 64 = ones,
rows 65-127 = zero).  scores S[q,k] (f32r, K=128) -> max (DVE) -> exp+Z (ACT)
-> attn = E*(1/Z) (GpSimd).  Transposed path: ST[k,q] = kct1^T . qctx gets the
-max bias via row 64 of the matmul; exp -> ET fp16; PV matmul against V with a
ones column appended so row 64 of the accumulator is Z_T; context normalized by
Z_T (independently consistent softmax - required because ST rounds differently
than S).
"""
import sys
sys.path.insert(0, '/opt/trn_rl_repo')
import numpy as np

import concourse.bass as bass
import concourse.bacc as bacc
import concourse.mybir as mybir
from concourse.bass_utils import run_bass_kernel_spmd
from concourse.tile import TileContext

F32 = mybir.dt.float32
F32R = mybir.dt.float32r
F16 = mybir.dt.float16
AX = mybir.AxisListType
ALU = mybir.AluOpType
ACTF = mybir.ActivationFunctionType

B, H, L, DK = 4, 8, 2048, 64
D = H * DK
HPC = 4
TPC = 1024
SQW = 1028

SCALE_MODE = "split"
_NC_CACHE = {}


def build_nc(ntaps):
    nc = bacc.Bacc(None, target_bir_lowering=False)

    sq = nc.declare_dram_parameter("sq", [D, SQW], F32, isOutput=False)
    sk = nc.declare_dram_parameter("sk", [D, SQW], F32, isOutput=False)
    wq = nc.declare_dram_parameter("wq", [D, ntaps * D], F32, isOutput=False)
    wk = nc.declare_dram_parameter("wk", [D, ntaps * D], F32, isOutput=False)
    rq = nc.declare_dram_parameter("rq", [TPC, D], F32, isOutput=False)
    rk = nc.declare_dram_parameter("rk", [TPC, D], F32, isOutput=False)
    vv = nc.declare_dram_parameter("v", [128, HPC * 16 * 65], F32, isOutput=False)
    idn = nc.declare_dram_parameter("ident", [128, 128], F32, isOutput=False)
    attn_o = nc.declare_dram_parameter("attn_o", [HPC, L, L], F32, isOutput=True)
    ctx_o = nc.declare_dram_parameter("ctx_o", [HPC, L, DK], F32, isOutput=True)

    with TileContext(nc) as tc:
        with tc.tile_pool(name="perm", bufs=1) as perm:
            ident = perm.tile([128, 128], F32, tag="ident")
            nc.sync.dma_start(ident[:], idn[:])
            ident16 = perm.tile([128, 128], F16, tag="ident16")
            nc.vector.tensor_copy(ident16[:], ident[:])
            qctx = perm.tile([128, HPC * L], F32R, tag="qctx")
            kct1 = perm.tile([128, HPC * L], F32R, tag="kct1")
            vbf = perm.tile([128, HPC * 16 * 65], F16, tag="vbf")
            ones_f = perm.tile([1, HPC * L], F32, tag="ones")
            nc.gpsimd.memset(qctx[:].bitcast(F32), 0.0)
            nc.gpsimd.memset(kct1[:].bitcast(F32), 0.0)
            nc.gpsimd.memset(ones_f[:], 1.0)
            nc.vector.tensor_copy(kct1[64:65, :], ones_f[:])

            # ---------- phase A/B: per-signal load + round + conv ----------
            with tc.tile_pool(name="stage", bufs=2) as stage, \
                 tc.tile_pool(name="cps", bufs=2, space="PSUM") as cps, \
                 tc.tile_pool(name="cpt", bufs=2, space="PSUM") as cpt, \
                 tc.tile_pool(name="work", bufs=3) as wk_pool, \
                 tc.tile_pool(name="resid", bufs=4) as resid:
                for i4 in range(4):
                    sv = stage.tile([128, 1040], F32, tag="stg_s")
                    nc.sync.dma_start(sv[:], vv[:, i4 * 1040:(i4 + 1) * 1040])
                    nc.vector.tensor_copy(vbf[:, i4 * 1040:(i4 + 1) * 1040], sv[:])

                jobs = [("q", sq, wq, rq, qctx), ("k", sk, wk, rk, kct1)]
                for name, src_s, src_w, rsrc, dst in jobs:
                  with tc.tile_pool(name=f"cperm{name}", bufs=1) as cperm:
                    sig_r = {}
                    w_r = {}
                    for ci in range(4):
                        st = stage.tile([128, SQW], F32, tag="stg_s")
                        nc.sync.dma_start(st[:], src_s[ci * 128:(ci + 1) * 128, :])
                        t = cperm.tile([128, SQW], F32R, tag=f"sr{ci}")
                        nc.vector.tensor_copy(t[:], st[:])
                        sig_r[ci] = t
                        sw = stage.tile([128, ntaps * D], F32, tag="stg_w")
                        nc.sync.dma_start(sw[:], src_w[ci * 128:(ci + 1) * 128, :])
                        tw = cperm.tile([128, ntaps * D], F32R, tag=f"wr{ci}")
                        nc.vector.tensor_copy(tw[:], sw[:])
                        w_r[ci] = tw
                    for it in range(8):
                        ps = cps.tile([128, D], F32, tag="convps")
                        nmm = 4 * ntaps
                        i_mm = 0
                        for ci in range(4):
                            for jj in range(ntaps):
                                nc.tensor.matmul(
                                    ps[:],
                                    sig_r[ci][:, it * 128 + jj:it * 128 + jj + 128],
                                    w_r[ci][:, jj * D:(jj + 1) * D],
                                    start=(i_mm == 0), stop=(i_mm == nmm - 1),
                                )
                                i_mm += 1
                        rt = resid.tile([128, D], F32, tag="rt")
                        nc.sync.dma_start(rt[:], rsrc[it * 128:(it + 1) * 128, :])
                        qc = wk_pool.tile([128, D], F32, tag="qc")
                        nc.vector.tensor_tensor(qc[:], ps[:], rt[:], ALU.add)
                        hh = it // 2
                        dview = dst[0:64, :].rearrange("p (x t g) -> p x t g", t=128, g=8)
                        for g2 in range(4):
                            pt = cpt.tile([128, 128], F32, tag="tp")
                            nc.tensor.transpose(pt[:], qc[:, g2 * 128:(g2 + 1) * 128], ident[:])
                            for sub in range(2):
                                g = g2 * 2 + sub
                                nc.vector.tensor_copy(
                                    dview[:, 2 * hh + (it % 2), :, g],
                                    pt[sub * 64:(sub + 1) * 64, :])

            # ---------- phase C: attention ----------
            with tc.tile_pool(name="ps_s", bufs=2, space="PSUM") as ps_s, \
                 tc.tile_pool(name="ps_st", bufs=2, space="PSUM") as ps_st, \
                 tc.tile_pool(name="ps_c", bufs=2, space="PSUM") as ps_c, \
                 tc.tile_pool(name="sbE", bufs=5) as sbE, \
                 tc.tile_pool(name="sbET", bufs=3) as sbET, \
                 tc.tile_pool(name="sbA", bufs=3) as sbA, \
                 tc.tile_pool(name="sbC", bufs=4) as sbC, \
                 tc.tile_pool(name="stats", bufs=10) as stats:
                def emit_st_block(hh, g4):
                    hb = hh * L
                    qb = hb + g4 * 512
                    ctxT = ps_c.tile([65, 512], F32, tag="ctx")
                    for kc in range(16):
                        stp = ps_st.tile([128, 512], F32, tag="st")
                        nc.tensor.matmul(
                            stp[:],
                            kct1[:, hb + kc * 128: hb + (kc + 1) * 128],
                            qctx[:, qb:qb + 512],
                            start=True, stop=True)
                        ET = sbET.tile([128, 512], F16, tag="ET")
                        nc.scalar.activation(ET[:], stp[:], ACTF.Exp)
                        nc.tensor.matmul(
                            ctxT[:],
                            vbf[:, (hh * 16 + kc) * 65:(hh * 16 + kc + 1) * 65],
                            ET[:],
                            start=(kc == 0), stop=(kc == 15))
                    rzT = sbC.tile([1, 512], F32, tag="rzT")
                    nc.vector.reciprocal(rzT[:], ctxT[64:65, :])
                    rzB = sbC.tile([64, 512], F32, tag="rzB")
                    nc.gpsimd.partition_broadcast(rzB[:], rzT[:])
                    cs = sbC.tile([64, 512], F32, tag="cs")
                    nc.vector.tensor_tensor(cs[:], ctxT[0:64, :], rzB[:], ALU.mult)
                    for i4 in range(4):
                        ct = ps_c.tile([128, 64], F32, tag="ctx")
                        nc.tensor.transpose(ct[:], cs[:, i4 * 128:(i4 + 1) * 128],
                                            ident[:64, :64])
                        cf = sbC.tile([128, 64], F32, tag="cf")
                        nc.scalar.copy(cf[:], ct[:])
                        qt = g4 * 4 + i4
                        nc.sync.dma_start(
                            ctx_o[hh, qt * 128:(qt + 1) * 128, :], cf[:])

                pending_st = None
                for hh in range(HPC):
                    hb = hh * L
                    for g4 in range(4):
                        zbA = stats.tile([128, 4], F32, tag="zbA")
                        zbB = stats.tile([128, 4], F32, tag="zbB")
                        E4 = []
                        for i4 in range(4):
                            qt = g4 * 4 + i4
                            q0 = hb + qt * 128
                            sA = ps_s.tile([128, 1024], F32, tag="s")
                            sB = ps_s.tile([128, 1024], F32, tag="s")
                            for kb in range(2):
                                nc.tensor.matmul(
                                    sA[:, kb * 512:(kb + 1) * 512],
                                    qctx[:, q0:q0 + 128],
                                    kct1[:, hb + kb * 512: hb + (kb + 1) * 512],
                                    start=True, stop=True)
                            for kb in range(2):
                                nc.tensor.matmul(
                                    sB[:, kb * 512:(kb + 1) * 512],
                                    qctx[:, q0:q0 + 128],
                                    kct1[:, hb + 1024 + kb * 512: hb + 1024 + (kb + 1) * 512],
                                    start=True, stop=True)
                            m1 = stats.tile([128, 1], F32, tag="m1")
                            m2 = stats.tile([128, 1], F32, tag="m2")
                            nc.vector.tensor_reduce(m1[:], sA[:], AX.X, ALU.max)
                            nc.vector.tensor_reduce(m2[:], sB[:], AX.X, ALU.max)
                            mm = stats.tile([128, 1], F32, tag="mm")
                            nc.vector.tensor_tensor(mm[:], m1[:], m2[:], ALU.max)
                            negm = stats.tile([128, 1], F16, tag="negm")
                            nc.vector.tensor_scalar_mul(negm[:], mm[:], -1.0)
                            negm32 = stats.tile([128, 1], F32, tag="negm32")
                            nc.vector.tensor_copy(negm32[:], negm[:])
                            ptn = ps_c.tile([1, 128], F16, tag="ctx")
                            nc.tensor.transpose(ptn[:], negm[:], ident16[:])
                            nc.vector.tensor_copy(qctx[64:65, q0:q0 + 128], ptn[:])
                            E = sbE.tile([128, L], F16, tag="E")
                            nc.scalar.activation(E[:, :1024], sA[:], ACTF.Exp,
                                                 bias=negm32[:], scale=1.0,
                                                 accum_out=zbA[:, i4:i4 + 1])
                            nc.scalar.activation(E[:, 1024:], sB[:], ACTF.Exp,
                                                 bias=negm32[:], scale=1.0,
                                                 accum_out=zbB[:, i4:i4 + 1])
                            E4.append(E)
                        rzb = stats.tile([128, 4], F32, tag="rzb")
                        nc.vector.tensor_tensor(rzb[:], zbA[:], zbB[:], ALU.add)
                        nc.vector.reciprocal(rzb[:], rzb[:])
                        for i4 in range(4):
                            qt = g4 * 4 + i4
                            at = sbA.tile([128, L], F32, tag="at")
                            if SCALE_MODE == "gpsimd":
                                nc.gpsimd.tensor_scalar(at[:], E4[i4][:],
                                                        rzb[:, i4:i4 + 1], None, ALU.mult)
                            else:
                                nc.vector.tensor_scalar_mul(at[:, :1024], E4[i4][:, :1024],
                                                            rzb[:, i4:i4 + 1])
                                nc.scalar.activation(at[:, 1024:], E4[i4][:, 1024:],
                                                     ACTF.Copy, scale=rzb[:, i4:i4 + 1])
                            nc.sync.dma_start(attn_o[hh, qt * 128:(qt + 1) * 128, :], at[:])
                        # transposed path lags one group so the latency-critical
                        # scores/max/negm/exp chain isn't queued behind it on PE
                        if pending_st is not None:
                            emit_st_block(*pending_st)
                        pending_st = (hh, g4)
                if pending_st is not None:
                    emit_st_block(*pending_st)
    nc.compile()
    return nc


def _get_nc(ntaps):
    if ntaps not in _NC_CACHE:
        _NC_CACHE[ntaps] = build_nc(ntaps)
    return _NC_CACHE[ntaps]


def _prep_inputs(Q, K, V, conv_q, conv_k, w):
    f_s = np.array([2.0, 4.0], dtype=w.dtype)
    ind = int(np.argmax(f_s * w))
    chosen = (2, 4)[ind]
    taps = list(range(chosen))
    ntaps = len(taps)

    ident = np.eye(128, dtype=np.float32)
    in_maps = []
    for core in range(8):
        bi, th = core // 2, core % 2
        t0 = th * TPC
        h0 = HPC * th
        qsig = Q[bi].reshape(D, 2 * TPC)
        ksig = K[bi].reshape(D, 2 * TPC)

        def slc(sig):
            out = np.zeros((D, SQW), dtype=np.float32)
            lo, hi = t0 - 2, t0 + 1026
            s_lo, s_hi = max(lo, 0), min(hi, 2 * TPC)
            out[:, s_lo - lo:s_hi - lo] = sig[:, s_lo:s_hi]
            return out

        wq_h = np.ascontiguousarray(
            conv_q[:, :, taps].transpose(0, 2, 1).reshape(D, ntaps * D)) * np.float32(0.125)
        wk_h = np.ascontiguousarray(
            conv_k[:, :, taps].transpose(0, 2, 1).reshape(D, ntaps * D))
        rq_h = Q[bi].reshape(2 * TPC, D)[t0:t0 + TPC] * np.float32(0.125)
        rk_h = np.ascontiguousarray(K[bi].reshape(2 * TPC, D)[t0:t0 + TPC])
        v4 = V[bi, h0:h0 + HPC].reshape(HPC, 16, 128, DK).transpose(2, 0, 1, 3)
        v_h = np.concatenate(
            [v4, np.ones((128, HPC, 16, 1), dtype=np.float32)], axis=3
        ).reshape(128, HPC * 16 * 65)
        in_maps.append({
            "sq": slc(qsig), "sk": slc(ksig),
            "wq": wq_h.astype(np.float32), "wk": wk_h.astype(np.float32),
            "rq": rq_h.astype(np.float32), "rk": rk_h,
            "v": np.ascontiguousarray(v_h), "ident": ident,
        })
    return in_maps, ntaps


def kernel(Q, K, V, attn_mask, conv_q, conv_k, w, _trace=False):
    Q, K, V = np.asarray(Q), np.asarray(K), np.asarray(V)
    conv_q, conv_k, w = np.asarray(conv_q), np.asarray(conv_k), np.asarray(w)
    in_maps, ntaps = _prep_inputs(Q, K, V, conv_q, conv_k, w)
    nc = _get_nc(ntaps)
    res = run_bass_kernel_spmd(nc, in_maps, core_ids=list(range(8)), trace=_trace)
    context = np.empty((B, H, L, DK), dtype=np.float32)
    attn = np.empty((B, H, L, L), dtype=np.float32)
    for core in range(8):
        bi, th = core // 2, core % 2
        h0 = HPC * th
        attn[bi, h0:h0 + HPC] = res.results[core]["attn_o"]
        context[bi, h0:h0 + HPC] = res.results[core]["ctx_o"]
    kernel._last_exec_time_ns = res.exec_time_ns
    return context, attn
